# revision 21
# baseline (speedup 1.0000x reference)
"""Trainium2 Bass kernel for the stacked-Chebyshev locally-connected net.

Reference computation (B=256, k=6250, d*d=4096, O=10):
    x1 = z @ (mask*T1).T
    x2 = 2*(z @ (mask*T2).T)*x1 - T0
    x3 = 2*(z @ (mask*T3).T)*x2 - x1
    out = x3 @ C_w.T + C_b

The mask is a locally-connected conv pattern: 16x16 patch, stride 2, 25x25
positions, stacked 10x.  A 4(i)x3(j) block of positions x 10 stacks = 120
k-columns whose patch union is 22x20 = 440 pixels.

Sharding (SPMD-uniform across 8 cores): the 24x24 position subgrid = 48
blocks, 6 per core (one run of 4 + one run of 2 along j).  Row 24 /
col 24 (49 positions) are covered by 8 overlapping 7-position strips
(support 16x28 = 448 = 4 chunks of 112), one per core; the 7
doubly-covered positions get C_w x 0.5 so the host-side sum over cores
stays correct.

The kernel is DMA-bound (~3.4MB/core at ~190-225 B/ns effective), so z
for a run of tiles is shipped ONCE as shared 22x5-column groups (110
pixels = one contraction chunk): adjacent tiles overlap 14 of 20 support
columns, and the shared grid turns 16(run4)+8(run2) per-tile z chunks
into 8+6 shared groups (-562KB), at the cost of 5 instead of 4 chunks
for the non-first tiles of a run (weights zero-padded to the group grid,
+316KB, +12 matmuls).

Execution: all tw pieces ride ONE need-ordered sync/HWDGE queue so units
complete sequentially at the aggregate rate; cwt rides gpsimd in
parallel.  Warmup matmuls + post-strip fillers keep the PE busy ~4us
continuously so the HAM clock-gate latches 2.4GHz early (re-throttle
needs ~3.4us of *continuous* idle, which the later inter-tile waits
never reach).  ACT does the x1 PSUM->SBUF copies, DVE the recurrence
products, and x3 = m3 - x1 is folded into the projection as psum +=
C*m3 + (-C)*x1.  The output accumulates in two PSUM halves; the first
drains mid-kernel, hidden under the remaining stream.
"""

import numpy as np

import concourse.bass as bass
import concourse.mybir as mybir
import concourse.tile as tile
from concourse import bacc
from concourse.bass_utils import run_bass_kernel_spmd

F32 = mybir.dt.float32
F32R = mybir.dt.float32r
F16 = mybir.dt.float16

B = 256          # batch
O = 10           # output classes
D = 64           # image side
N_CORES = 8
STACKS = 10

N_UNITS = 7
# strip start positions along the edges; position (24,24) sits in H-strip 18
H_STARTS = (0, 6, 12, 18)    # cores 0-3: (24, j0..j0+6)
V_STARTS = (0, 6, 12, 17)    # cores 4-7: (i0..i0+6, 24)
# positions covered by two strips -> C_w * 0.5
DUP_POS = {(24, 6), (24, 12), (24, 18), (6, 24), (12, 24), (17, 24), (18, 24)}

ZWS_W = 4 * B + 3 * 4 * 70           # 1864 cols: zs | ws
CWT_W = N_UNITS * 2 * O + N_UNITS    # 147: [+C | -C] x 7 units, then -T0 col
N_WARM = 70
N_FILL = 16

# chunk-group sets per tile Delta within a run (22x5-col z groups)
CHUNK_SETS = {4: [range(0, 4), range(1, 6), range(2, 7), range(3, 8)],
              2: [range(0, 4), range(1, 6)]}


def _tw_layout():
    """Column layout of the tw tensor: per full-unit consumption-ordered
    blocks [new z groups | weight block].  Returns (units, total_cols)."""
    pos = 0
    units = []
    for L in (4, 2):
        zcol = {}
        for delta, gs in enumerate(CHUNK_SETS[L]):
            blk0 = pos
            for g in gs:
                if g not in zcol:
                    zcol[g] = pos
                    pos += B
            nch = len(gs)
            wbase = pos
            pos += 3 * nch * 120
            units.append(dict(blk0=blk0, blkend=pos, nch=nch, wbase=wbase,
                              zcols=[zcol[g] for g in gs], L=L, delta=delta))
    return units, pos


TW_UNITS, TW_COLS = _tw_layout()


def _full_runs(c):
    """[(band, jb0, L), ...] for core c: one run of 4 + one run of 2."""
    return [(c // 2, 4 * (c % 2), 4), (4 + c // 4, 2 * (c % 4), 2)]


def _strip_pos(c):
    if c < 4:
        return [(24, H_STARTS[c] + q) for q in range(7)]
    return [(V_STARTS[c - 4] + q, 24) for q in range(7)]


def _kcols(pos):
    return np.array(
        [s * 625 + i * 25 + j for s in range(STACKS) for (i, j) in pos],
        dtype=np.int64)


def _strip_sup(c):
    """col/row-major support (448 d-indices) of core c's strip, chunked 4x112."""
    if c < 4:
        j0 = H_STARTS[c]
        r = np.arange(16)
        cl = np.arange(28)
        return ((48 + r)[None, :] * D + 2 * j0 + cl[:, None]).ravel()
    i0 = V_STARTS[c - 4]
    r = np.arange(28)
    cl = np.arange(16)
    return ((2 * i0 + r)[:, None] * D + 48 + cl[None, :]).ravel()


def _group_sup(band, jb0, g):
    """col-major pixel indices (110) of z group g of run (band, jb0);
    clamped: out-of-band columns carry garbage that zero weights kill."""
    r = np.arange(22)
    cl = np.arange(5)
    sup = ((8 * band + r)[None, :] * D + 6 * jb0 + 5 * g + cl[:, None]).ravel()
    return np.minimum(sup, D * D - 1)


def _build_nc():
    nc = bacc.Bacc(
        "TRN2", target_bir_lowering=False, debug=False, num_devices=N_CORES,
        enable_partition_id=False, monotonic_sem_count=0,
    )
    zws = nc.dram_tensor("zws", [112, ZWS_W], F16, kind="ExternalInput").ap()
    tw = nc.dram_tensor("tw", [110, TW_COLS], F16, kind="ExternalInput").ap()
    cwt = nc.dram_tensor("cwt", [128, CWT_W], F32R, kind="ExternalInput").ap()
    out = nc.dram_tensor("out", [2 * O, B], F32, kind="ExternalOutput").ap()

    with tile.TileContext(nc) as tc:
        with (
            tc.tile_pool(name="dpool", bufs=1) as dpool,
            tc.tile_pool(name="xpool", bufs=3) as xpool,
            tc.tile_pool(name="ppool", bufs=5, space="PSUM") as ppool,
            tc.tile_pool(name="opool", bufs=1, space="PSUM") as opool,
        ):
            zws_sb = dpool.tile([112, ZWS_W], F16, tag="zws")
            tw_sb = dpool.tile([110, TW_COLS], F16, tag="tw")
            cw_sb = dpool.tile([128, CWT_W], F32R, tag="cw")

            # warmup tiles, memset by DVE (free at body start)
            wu_sb = dpool.tile([128, 16], F16, tag="wu")
            zu_sb = dpool.tile([128, 64], F16, tag="zu")
            nc.vector.memset(wu_sb[:], 0.0)
            nc.vector.memset(zu_sb[:], 0.0)

            # DMA schedule: one need-ordered queue (sync/HWDGE); cwt rides
            # gpsimd in parallel.  One piece per unit block; the last unit's
            # weights split per layer so the post-stream chain is short.
            nc.gpsimd.dma_start(cw_sb[:], cwt[:])
            for U in TW_UNITS:
                c0, c1 = U["blk0"], U["blkend"]
                nc.sync.dma_start(tw_sb[:, c0:c1], tw[:, c0:c1])
            # strip last: split so the final post-stream chain is one
            # 4-matmul layer + the recurrence tail
            ZA = 4 * B + 4 * 70
            ZB = ZA + 4 * 70
            nc.sync.dma_start(zws_sb[:, 0:ZA], zws[:, 0:ZA])
            nc.sync.dma_start(zws_sb[:, ZA:ZB], zws[:, ZA:ZB])
            nc.sync.dma_start(zws_sb[:, ZB:], zws[:, ZB:])

            pwarm_t = opool.tile([16, 64], F32, tag="warm")
            pwarm = pwarm_t[:]
            psum_a_t = opool.tile([O, B], F32, tag="outa")
            psum_a = psum_a_t[:]
            psum_b_t = opool.tile([O, B], F32, tag="outb")
            psum_b = psum_b_t[:]
            for _ in range(N_WARM):
                nc.tensor.matmul(pwarm, wu_sb[:], zu_sb[:],
                                 start=True, stop=True)
            pending = []
            n_proj = 0
            # units 0-3 accumulate into psum_a (8 proj MMs), 4-6 into psum_b
            NPA = 8

            out_sb = dpool.tile([O, 2 * B], F32, tag="out")

            def flush_proj(last=False):
                nonlocal n_proj
                for cslice, rhs in pending:
                    n_proj += 1
                    tgt = psum_a if n_proj <= NPA else psum_b
                    nc.tensor.matmul(tgt, cslice, rhs,
                                     start=(n_proj == 1 or n_proj == NPA + 1),
                                     stop=(n_proj == NPA or
                                           (last and n_proj == 2 * N_UNITS)))
                    if n_proj == NPA:
                        # units 0-3 done: drain psum_a early, hidden under
                        # the remaining tiles' compute/DMA
                        nc.vector.tensor_copy(out_sb[:, 0:B], psum_a)
                        nc.sync.dma_start(out[0:O, :], out_sb[:, 0:B])
                pending.clear()

            for ui, u in enumerate([1, 2, 3, 4, 5, 6, 0]):
                if ui == 1:
                    # fillers: keep the PE-busy stretch continuous until the
                    # HAM clock-gate latches 8/8
                    for _ in range(N_FILL):
                        nc.tensor.matmul(pwarm, wu_sb[:], zu_sb[:],
                                         start=True, stop=True)
                if u == 0:
                    nch, cols = 4, 70
                else:
                    TU = TW_UNITS[u - 1]
                    nch, cols = TU["nch"], 120
                xs = {}
                for li in range(3):
                    if li == 2:
                        flush_proj()
                    p = ppool.tile([cols, B], F32, tag="ps")
                    for ch in range(nch):
                        if u == 0:
                            lhsT = zws_sb[:, 4 * B + (li * nch + ch) * cols:
                                          4 * B + (li * nch + ch + 1) * cols]
                            rhs = zws_sb[:, ch * B:(ch + 1) * B]
                        else:
                            w0 = TU["wbase"] + (li * nch + ch) * cols
                            lhsT = tw_sb[:, w0:w0 + cols]
                            z0 = TU["zcols"][ch]
                            rhs = tw_sb[:, z0:z0 + B]
                        nc.tensor.matmul(p[:], lhsT, rhs,
                                         start=(ch == 0), stop=(ch == nch - 1))
                    if li == 0:
                        x1 = xpool.tile([cols, B], F32R, tag="x1")
                        nc.scalar.copy(x1[:], p[:])
                        xs["x1"] = x1
                        pending.append(
                            (cw_sb[0:cols, u * 2 * O + O:u * 2 * O + 2 * O],
                             x1[:]))
                    elif li == 1:
                        m2 = xpool.tile([cols, B], F32, tag="m2")
                        x2 = xpool.tile([cols, B], F32, tag="x2")
                        nc.vector.tensor_mul(m2[:], p[:], xs["x1"][:])
                        nc.vector.tensor_scalar_add(
                            x2[:], m2[:],
                            cw_sb[0:cols, 2 * O * N_UNITS + u:
                                  2 * O * N_UNITS + u + 1].bitcast(F32))
                        xs["x2"] = x2
                    else:
                        m3 = xpool.tile([cols, B], F32R, tag="m3")
                        nc.vector.tensor_mul(m3[:], p[:], xs["x2"][:])
                        pending.append(
                            (cw_sb[0:cols, u * 2 * O:u * 2 * O + O],
                             m3[:]))
            flush_proj(last=True)

            nc.scalar.copy(out_sb[:, B:2 * B], psum_b)
            nc.scalar.dma_start(out[O:2 * O, :], out_sb[:, B:2 * B])

    nc.compile()
    return nc


_NC = None


def _get_nc():
    global _NC
    if _NC is None:
        _NC = _build_nc()
    return _NC


def _prepare_in_maps(z, T1, T2, T3, T0, C_w, mask):
    z = np.ascontiguousarray(np.asarray(z, dtype=np.float32).reshape(B, D * D))
    T1 = np.asarray(T1, dtype=np.float32)
    T2 = np.asarray(T2, dtype=np.float32)
    T3 = np.asarray(T3, dtype=np.float32)
    T0 = np.asarray(T0, dtype=np.float32)
    C_w = np.asarray(C_w, dtype=np.float32)
    mask = np.asarray(mask, dtype=np.float32)

    zT16 = np.ascontiguousarray(z.T).astype(np.float16)   # [4096, 256]
    Ts = (T1, T2, T3)
    scales = (1.0, 2.0, 2.0)

    def wpack(kcols, sup, K, nch, cols):
        wg = np.empty((K, 3, nch, cols), np.float16)
        for li, (T, sc) in enumerate(zip(Ts, scales)):
            A = (sc * T[np.ix_(kcols, sup)] * mask[np.ix_(kcols, sup)]).T
            wg[:, li] = A.reshape(nch, K, cols).transpose(1, 0, 2)
        return wg.reshape(K, 3 * nch * cols)

    in_maps = []
    for c in range(N_CORES):
        m = {}
        cwt = np.zeros((128, CWT_W), np.float32)

        # unit 0: strip -> zws
        spos = _strip_pos(c)
        skcols = _kcols(spos)
        ssup = _strip_sup(c)
        zws = np.empty((112, ZWS_W), np.float16)
        zws[:, 0:4 * B] = (zT16[ssup].reshape(4, 112, B)
                           .transpose(1, 0, 2).reshape(112, 4 * B))
        zws[:, 4 * B:] = wpack(skcols, ssup, 112, 4, 70)
        m["zws"] = np.ascontiguousarray(zws)
        cw_fac = np.array([0.5 if (i, j) in DUP_POS else 1.0
                           for s in range(STACKS) for (i, j) in spos])
        cs = (C_w[:, skcols] * cw_fac[None, :]).T          # [70, O]
        cwt[0:70, 0:O] = cs
        cwt[0:70, O:2 * O] = -cs
        cwt[0:70, 2 * O * N_UNITS + 0] = -T0[skcols]

        # units 1-6: shared z groups + per-tile weight blocks
        tw = np.empty((110, TW_COLS), np.float16)
        uidx = 1
        ui = 0
        for band, jb0, L in _full_runs(c):
            written = set()
            for delta, gs in enumerate(CHUNK_SETS[L]):
                TU = TW_UNITS[ui]
                sups = []
                for gi, g in enumerate(gs):
                    sg = _group_sup(band, jb0, g)
                    sups.append(sg)
                    if g not in written:
                        written.add(g)
                        zc = TU["zcols"][gi]
                        tw[:, zc:zc + B] = zT16[sg]
                pos = [(4 * band + a, 3 * (jb0 + delta) + b)
                       for a in range(4) for b in range(3)]
                kcols = _kcols(pos)
                sup = np.concatenate(sups)
                wb = TU["wbase"]
                tw[:, wb:wb + 3 * TU["nch"] * 120] = wpack(
                    kcols, sup, 110, TU["nch"], 120)
                cs = C_w[:, kcols].T                       # [120, O]
                cwt[0:120, uidx * 2 * O:uidx * 2 * O + O] = cs
                cwt[0:120, uidx * 2 * O + O:uidx * 2 * O + 2 * O] = -cs
                cwt[0:120, 2 * O * N_UNITS + uidx] = -T0[kcols]
                uidx += 1
                ui += 1
        m["tw"] = np.ascontiguousarray(tw)
        m["cwt"] = cwt
        in_maps.append(m)
    return in_maps


def kernel(z, T1, T2, T3, T0, C_w, C_b, mask):
    nc = _get_nc()
    in_maps = _prepare_in_maps(z, T1, T2, T3, T0, C_w, mask)
    res = run_bass_kernel_spmd(nc, in_maps, core_ids=list(range(N_CORES)))
    total = np.zeros((O, B), np.float32)
    for c in range(N_CORES):
        o = res.results[c]["out"]
        total += o[0:O] + o[O:2 * O]
    C_b = np.asarray(C_b, dtype=np.float32)
    return (total.T + C_b).astype(np.float32)


# revision 22
# speedup vs baseline: 1.0077x; 1.0077x over previous
"""Trainium2 Bass kernel for the stacked-Chebyshev locally-connected net.

Reference computation (B=256, k=6250, d*d=4096, O=10):
    x1 = z @ (mask*T1).T
    x2 = 2*(z @ (mask*T2).T)*x1 - T0
    x3 = 2*(z @ (mask*T3).T)*x2 - x1
    out = x3 @ C_w.T + C_b

The mask is a locally-connected conv pattern: 16x16 patch, stride 2, 25x25
positions, stacked 10x.  A 4(i)x3(j) block of positions x 10 stacks = 120
k-columns whose patch union is 22x20 = 440 pixels.

Sharding (SPMD-uniform across 8 cores): the 24x24 position subgrid = 48
blocks, 6 per core (one run of 4 + one run of 2 along j).  Row 24 /
col 24 (49 positions) are covered by 8 overlapping 7-position strips
(support 16x28 = 448 = 4 chunks of 112), one per core; the 7
doubly-covered positions get C_w x 0.5 so the host-side sum over cores
stays correct.

The kernel is DMA-bound (~3.4MB/core at ~190-225 B/ns effective), so z
for a run of tiles is shipped ONCE as shared 22x5-column groups (110
pixels = one contraction chunk): adjacent tiles overlap 14 of 20 support
columns, and the shared grid turns 16(run4)+8(run2) per-tile z chunks
into 8+6 shared groups (-562KB), at the cost of 5 instead of 4 chunks
for the non-first tiles of a run (weights zero-padded to the group grid,
+316KB, +12 matmuls).

Execution: all tw pieces ride ONE need-ordered sync/HWDGE queue so units
complete sequentially at the aggregate rate; cwt rides gpsimd in
parallel.  Warmup matmuls + post-strip fillers keep the PE busy ~4us
continuously so the HAM clock-gate latches 2.4GHz early (re-throttle
needs ~3.4us of *continuous* idle, which the later inter-tile waits
never reach).  ACT does the x1 PSUM->SBUF copies, DVE the recurrence
products, and x3 = m3 - x1 is folded into the projection as psum +=
C*m3 + (-C)*x1.  The output accumulates in two PSUM halves; the first
drains mid-kernel, hidden under the remaining stream.
"""

import numpy as np

import concourse.bass as bass
import concourse.mybir as mybir
import concourse.tile as tile
from concourse import bacc
from concourse.bass_utils import run_bass_kernel_spmd

F32 = mybir.dt.float32
F32R = mybir.dt.float32r
F16 = mybir.dt.float16

B = 256          # batch
O = 10           # output classes
D = 64           # image side
N_CORES = 8
STACKS = 10

N_UNITS = 7
# strip start positions along the edges; position (24,24) sits in H-strip 18
H_STARTS = (0, 6, 12, 18)    # cores 0-3: (24, j0..j0+6)
V_STARTS = (0, 6, 12, 17)    # cores 4-7: (i0..i0+6, 24)
# positions covered by two strips -> C_w * 0.5
DUP_POS = {(24, 6), (24, 12), (24, 18), (6, 24), (12, 24), (17, 24), (18, 24)}

ZWS_W = 4 * B + 3 * 4 * 70           # 1864 cols: zs | ws
CWT_W = N_UNITS * 2 * O + N_UNITS    # 147: [+C | -C] x 7 units, then -T0 col
N_WARM = 40
N_FILL = 16

# chunk-group sets per tile within a run (22x5-col z groups), emitted in
# REVERSED delta order so the final unit has 4 chunks and a small block
CHUNK_SETS = {4: [range(3, 8), range(2, 7), range(1, 6), range(0, 4)],
              2: [range(1, 6), range(0, 4)]}
DELTAS = {4: [3, 2, 1, 0], 2: [1, 0]}


def _tw_layout():
    """Column layout of the tw tensor: per full-unit consumption-ordered
    blocks [new z groups | weight block].  Returns (units, total_cols)."""
    pos = 0
    units = []
    for L in (4, 2):
        zcol = {}
        for delta, gs in zip(DELTAS[L], CHUNK_SETS[L]):
            blk0 = pos
            for g in gs:
                if g not in zcol:
                    zcol[g] = pos
                    pos += B
            nch = len(gs)
            wbase = pos
            pos += 3 * nch * 120
            units.append(dict(blk0=blk0, blkend=pos, nch=nch, wbase=wbase,
                              zcols=[zcol[g] for g in gs], L=L, delta=delta))
    return units, pos


TW_UNITS, TW_COLS = _tw_layout()


def _full_runs(c):
    """[(band, jb0, L), ...] for core c: one run of 4 + one run of 2."""
    return [(c // 2, 4 * (c % 2), 4), (4 + c // 4, 2 * (c % 4), 2)]


def _strip_pos(c):
    if c < 4:
        return [(24, H_STARTS[c] + q) for q in range(7)]
    return [(V_STARTS[c - 4] + q, 24) for q in range(7)]


def _kcols(pos):
    return np.array(
        [s * 625 + i * 25 + j for s in range(STACKS) for (i, j) in pos],
        dtype=np.int64)


def _strip_sup(c):
    """col/row-major support (448 d-indices) of core c's strip, chunked 4x112."""
    if c < 4:
        j0 = H_STARTS[c]
        r = np.arange(16)
        cl = np.arange(28)
        return ((48 + r)[None, :] * D + 2 * j0 + cl[:, None]).ravel()
    i0 = V_STARTS[c - 4]
    r = np.arange(28)
    cl = np.arange(16)
    return ((2 * i0 + r)[:, None] * D + 48 + cl[None, :]).ravel()


def _group_sup(band, jb0, g):
    """col-major pixel indices (110) of z group g of run (band, jb0);
    clamped: out-of-band columns carry garbage that zero weights kill."""
    r = np.arange(22)
    cl = np.arange(5)
    sup = ((8 * band + r)[None, :] * D + 6 * jb0 + 5 * g + cl[:, None]).ravel()
    return np.minimum(sup, D * D - 1)


def _build_nc():
    nc = bacc.Bacc(
        "TRN2", target_bir_lowering=False, debug=False, num_devices=N_CORES,
        enable_partition_id=False, monotonic_sem_count=0,
    )
    zws = nc.dram_tensor("zws", [112, ZWS_W], F16, kind="ExternalInput").ap()
    tw = nc.dram_tensor("tw", [110, TW_COLS], F16, kind="ExternalInput").ap()
    cwt = nc.dram_tensor("cwt", [128, CWT_W], F32R, kind="ExternalInput").ap()
    out = nc.dram_tensor("out", [2 * O, B], F32, kind="ExternalOutput").ap()

    with tile.TileContext(nc) as tc:
        with (
            tc.tile_pool(name="dpool", bufs=1) as dpool,
            tc.tile_pool(name="xpool", bufs=3) as xpool,
            tc.tile_pool(name="ppool", bufs=5, space="PSUM") as ppool,
            tc.tile_pool(name="opool", bufs=1, space="PSUM") as opool,
        ):
            zws_sb = dpool.tile([112, ZWS_W], F16, tag="zws")
            tw_sb = dpool.tile([110, TW_COLS], F16, tag="tw")
            cw_sb = dpool.tile([128, CWT_W], F32R, tag="cw")

            # warmup tiles, memset by DVE (free at body start)
            wu_sb = dpool.tile([128, 16], F16, tag="wu")
            zu_sb = dpool.tile([128, 64], F16, tag="zu")
            nc.vector.memset(wu_sb[:], 0.0)
            nc.vector.memset(zu_sb[:], 0.0)

            # DMA schedule: one need-ordered queue (sync/HWDGE); cwt rides
            # gpsimd in parallel.  One piece per unit block; the last unit's
            # weights split per layer so the post-stream chain is short.
            nc.gpsimd.dma_start(cw_sb[:], cwt[:])
            ZA = 4 * B + 4 * 70
            nc.sync.dma_start(zws_sb[:, 0:ZA], zws[:, 0:ZA])
            nc.sync.dma_start(zws_sb[:, ZA:], zws[:, ZA:])
            for ui, U in enumerate(TW_UNITS):
                c0, c1 = U["blk0"], U["blkend"]
                if ui < len(TW_UNITS) - 1:
                    nc.sync.dma_start(tw_sb[:, c0:c1], tw[:, c0:c1])
                else:
                    # final block split per layer: the post-stream chain is
                    # one 4-matmul layer + the recurrence tail
                    nch = U["nch"]
                    cm1 = U["wbase"] + nch * 120
                    cm2 = U["wbase"] + 2 * nch * 120
                    nc.sync.dma_start(tw_sb[:, c0:cm1], tw[:, c0:cm1])
                    nc.sync.dma_start(tw_sb[:, cm1:cm2], tw[:, cm1:cm2])
                    nc.sync.dma_start(tw_sb[:, cm2:c1], tw[:, cm2:c1])

            pwarm_t = opool.tile([16, 64], F32, tag="warm")
            pwarm = pwarm_t[:]
            psum_a_t = opool.tile([O, B], F32, tag="outa")
            psum_a = psum_a_t[:]
            psum_b_t = opool.tile([O, B], F32, tag="outb")
            psum_b = psum_b_t[:]
            for _ in range(N_WARM):
                nc.tensor.matmul(pwarm, wu_sb[:], zu_sb[:],
                                 start=True, stop=True)
            pending = []
            n_proj = 0
            # units 0-3 accumulate into psum_a (8 proj MMs), 4-6 into psum_b
            NPA = 8

            out_sb = dpool.tile([O, 2 * B], F32, tag="out")

            def flush_proj(last=False):
                nonlocal n_proj
                for cslice, rhs in pending:
                    n_proj += 1
                    tgt = psum_a if n_proj <= NPA else psum_b
                    nc.tensor.matmul(tgt, cslice, rhs,
                                     start=(n_proj == 1 or n_proj == NPA + 1),
                                     stop=(n_proj == NPA or
                                           (last and n_proj == 2 * N_UNITS)))
                    if n_proj == NPA:
                        # units 0-3 done: drain psum_a early, hidden under
                        # the remaining tiles' compute/DMA
                        nc.vector.tensor_copy(out_sb[:, 0:B], psum_a)
                        nc.sync.dma_start(out[0:O, :], out_sb[:, 0:B])
                pending.clear()

            for ui, u in enumerate([0, 1, 2, 3, 4, 5, 6]):
                if ui == 1:
                    # fillers: keep the PE-busy stretch continuous until the
                    # HAM clock-gate latches 8/8
                    for _ in range(N_FILL):
                        nc.tensor.matmul(pwarm, wu_sb[:], zu_sb[:],
                                         start=True, stop=True)
                if u == 0:
                    nch, cols = 4, 70
                else:
                    TU = TW_UNITS[u - 1]
                    nch, cols = TU["nch"], 120
                xs = {}
                for li in range(3):
                    if li == 2:
                        flush_proj()
                    p = ppool.tile([cols, B], F32, tag="ps")
                    for ch in range(nch):
                        if u == 0:
                            lhsT = zws_sb[:, 4 * B + (li * nch + ch) * cols:
                                          4 * B + (li * nch + ch + 1) * cols]
                            rhs = zws_sb[:, ch * B:(ch + 1) * B]
                        else:
                            w0 = TU["wbase"] + (li * nch + ch) * cols
                            lhsT = tw_sb[:, w0:w0 + cols]
                            z0 = TU["zcols"][ch]
                            rhs = tw_sb[:, z0:z0 + B]
                        nc.tensor.matmul(p[:], lhsT, rhs,
                                         start=(ch == 0), stop=(ch == nch - 1))
                    if li == 0:
                        x1 = xpool.tile([cols, B], F32R, tag="x1")
                        nc.scalar.copy(x1[:], p[:])
                        xs["x1"] = x1
                        pending.append(
                            (cw_sb[0:cols, u * 2 * O + O:u * 2 * O + 2 * O],
                             x1[:]))
                    elif li == 1:
                        m2 = xpool.tile([cols, B], F32, tag="m2")
                        x2 = xpool.tile([cols, B], F32, tag="x2")
                        nc.vector.tensor_mul(m2[:], p[:], xs["x1"][:])
                        nc.vector.tensor_scalar_add(
                            x2[:], m2[:],
                            cw_sb[0:cols, 2 * O * N_UNITS + u:
                                  2 * O * N_UNITS + u + 1].bitcast(F32))
                        xs["x2"] = x2
                    else:
                        m3 = xpool.tile([cols, B], F32R, tag="m3")
                        nc.vector.tensor_mul(m3[:], p[:], xs["x2"][:])
                        pending.append(
                            (cw_sb[0:cols, u * 2 * O:u * 2 * O + O],
                             m3[:]))
            flush_proj(last=True)

            nc.vector.tensor_copy(out_sb[:, B:2 * B], psum_b)
            nc.scalar.dma_start(out[O:2 * O, :], out_sb[:, B:2 * B])

    nc.compile()
    return nc


_NC = None


def _get_nc():
    global _NC
    if _NC is None:
        _NC = _build_nc()
    return _NC


def _prepare_in_maps(z, T1, T2, T3, T0, C_w, mask):
    z = np.ascontiguousarray(np.asarray(z, dtype=np.float32).reshape(B, D * D))
    T1 = np.asarray(T1, dtype=np.float32)
    T2 = np.asarray(T2, dtype=np.float32)
    T3 = np.asarray(T3, dtype=np.float32)
    T0 = np.asarray(T0, dtype=np.float32)
    C_w = np.asarray(C_w, dtype=np.float32)
    mask = np.asarray(mask, dtype=np.float32)

    zT16 = np.ascontiguousarray(z.T).astype(np.float16)   # [4096, 256]
    Ts = (T1, T2, T3)
    scales = (1.0, 2.0, 2.0)

    def wpack(kcols, sup, K, nch, cols):
        wg = np.empty((K, 3, nch, cols), np.float16)
        for li, (T, sc) in enumerate(zip(Ts, scales)):
            A = (sc * T[np.ix_(kcols, sup)] * mask[np.ix_(kcols, sup)]).T
            wg[:, li] = A.reshape(nch, K, cols).transpose(1, 0, 2)
        return wg.reshape(K, 3 * nch * cols)

    in_maps = []
    for c in range(N_CORES):
        m = {}
        cwt = np.zeros((128, CWT_W), np.float32)

        # unit 0: strip -> zws
        spos = _strip_pos(c)
        skcols = _kcols(spos)
        ssup = _strip_sup(c)
        zws = np.empty((112, ZWS_W), np.float16)
        zws[:, 0:4 * B] = (zT16[ssup].reshape(4, 112, B)
                           .transpose(1, 0, 2).reshape(112, 4 * B))
        zws[:, 4 * B:] = wpack(skcols, ssup, 112, 4, 70)
        m["zws"] = np.ascontiguousarray(zws)
        cw_fac = np.array([0.5 if (i, j) in DUP_POS else 1.0
                           for s in range(STACKS) for (i, j) in spos])
        cs = (C_w[:, skcols] * cw_fac[None, :]).T          # [70, O]
        cwt[0:70, 0:O] = cs
        cwt[0:70, O:2 * O] = -cs
        cwt[0:70, 2 * O * N_UNITS + 0] = -T0[skcols]

        # units 1-6: shared z groups + per-tile weight blocks
        tw = np.empty((110, TW_COLS), np.float16)
        uidx = 1
        ui = 0
        for band, jb0, L in _full_runs(c):
            written = set()
            for delta, gs in zip(DELTAS[L], CHUNK_SETS[L]):
                TU = TW_UNITS[ui]
                sups = []
                for gi, g in enumerate(gs):
                    sg = _group_sup(band, jb0, g)
                    sups.append(sg)
                    if g not in written:
                        written.add(g)
                        zc = TU["zcols"][gi]
                        tw[:, zc:zc + B] = zT16[sg]
                pos = [(4 * band + a, 3 * (jb0 + delta) + b)
                       for a in range(4) for b in range(3)]
                kcols = _kcols(pos)
                sup = np.concatenate(sups)
                wb = TU["wbase"]
                tw[:, wb:wb + 3 * TU["nch"] * 120] = wpack(
                    kcols, sup, 110, TU["nch"], 120)
                cs = C_w[:, kcols].T                       # [120, O]
                cwt[0:120, uidx * 2 * O:uidx * 2 * O + O] = cs
                cwt[0:120, uidx * 2 * O + O:uidx * 2 * O + 2 * O] = -cs
                cwt[0:120, 2 * O * N_UNITS + uidx] = -T0[kcols]
                uidx += 1
                ui += 1
        m["tw"] = np.ascontiguousarray(tw)
        m["cwt"] = cwt
        in_maps.append(m)
    return in_maps


def kernel(z, T1, T2, T3, T0, C_w, C_b, mask):
    nc = _get_nc()
    in_maps = _prepare_in_maps(z, T1, T2, T3, T0, C_w, mask)
    res = run_bass_kernel_spmd(nc, in_maps, core_ids=list(range(N_CORES)))
    total = np.zeros((O, B), np.float32)
    for c in range(N_CORES):
        o = res.results[c]["out"]
        total += o[0:O] + o[O:2 * O]
    C_b = np.asarray(C_b, dtype=np.float32)
    return (total.T + C_b).astype(np.float32)
